# revision 1
# baseline (speedup 1.0000x reference)
"""Trainium2 Bass kernel for nn_FFNNTransducerModel (RNN-T style transducer).

Strategy
--------
The output grid [B, T, U+1, V] is ragged: only t < enc_size[b], u <= tgt_size[b]
is nonzero (the reference multiplies by that mask).  We therefore:

  host:   - run the tiny prediction network (embedding + 2-layer MLP + Wp
            projection + bj1) in numpy -> per-(b,u) bias vector bvec[b,u,512]
          - enumerate valid work items (b, t_tile, u), load-balance them
            across the 8 NeuronCores into a fixed (S slots x C items) grid
          - gather each core's enc slices (transposed) and bias vectors into
            dense, uniform-shaped input arrays (SPMD: one program, per-core
            data)
  device: - enc projection:  encp_T[j, t] = We.T @ encT  (PE, N=512 streams)
          - per item:        h[jc] = relu(encp_T[jc] + bvec)   (DVE/ACT,
                             fused add+relu via tensor_scalar / activation)
          - joint GEMM:      psum[v, (4 items x t)] += wj2[jc].T @ h4[jc]
                             (4 items batched along free dim -> N=512)
          - epilogue:        out = psum + bj2 (per-partition bias), DMA out
  host:   - scatter item tiles (transposed) into the zero-initialized full
            output; invalid region stays exactly 0 like the reference.

The compiled program is input-independent (all data dependence lives in the
host-prepared arrays), so the NEFF is built once and cached.
"""

import math
import os
import sys
import types

import numpy as np

import concourse.bass as bass
import concourse.mybir as mybir
import concourse.tile as tile
from concourse import bass_utils

F32 = mybir.dt.float32
P = 128

# Model dims (fixed by the problem)
B, T, U, V = 8, 512, 64, 128
ENC, PRED, JOIN, EMB, H = 512, 256, 512, 128, 2
NU = U + 1  # 65

# Device grid: S enc-slice slots x C items per slot, items grouped by 4
S_SLOTS = 10
C_ITEMS = 8  # must be a multiple of 4
GROUP = 4
NGROUP = C_ITEMS // GROUP  # groups per slot

_CACHE = {}


def _install_ntff_hook():
    """The image's antenv lacks axon_hooks; shim it so trace=True works."""
    if "antenv.axon_hooks" in sys.modules:
        return
    mod = types.ModuleType("antenv.axon_hooks")
    _hook = [None]
    mod.set_axon_ntff_profile_hook = lambda h: _hook.__setitem__(0, h)
    mod.get_axon_ntff_profile_hook = lambda: _hook[0]
    sys.modules["antenv.axon_hooks"] = mod
    try:
        from trn_agent_boot.trn_boot import _ntff_profile_via_ctypes

        mod.set_axon_ntff_profile_hook(
            _ntff_profile_via_ctypes("/opt/axon/libaxon_pjrt.so")
        )
    except Exception:
        pass


def _split_excess_waits(nc, max_waits=1):
    """This container's walrus supports only one embedded sync-wait per
    instruction; split extras into standalone EventSemaphore waits placed
    immediately before the consumer on the same engine stream."""
    f = nc.m.functions[0]
    for blk in f.blocks:
        insts = list(blk.instructions)
        out = []
        changed = False
        for ins in insts:
            si = getattr(ins, "sync_info", None)
            if si is not None and si.on_wait is not None and len(si.on_wait) > max_waits:
                waits = list(si.on_wait)
                keep, excess = waits[:max_waits], waits[max_waits:]
                for j, w in enumerate(excess):
                    es = mybir.InstEventSemaphore(
                        name=f"{ins.name}_xw{j}",
                        engine=ins.engine,
                        sync_info=mybir.SyncInfo(on_wait=[w], on_update=[]),
                    )
                    out.append(es)
                si.on_wait = keep
                changed = True
            out.append(ins)
        if changed:
            blk.instructions = out
    return nc


def _build_nc(S, C):
    """Uniform SPMD program: all data dependence is in the input arrays."""
    NG = C // GROUP
    nc = bass.Bass()
    encT = nc.dram_tensor("encT", [S, ENC, P], F32, kind="ExternalInput")
    bvecs = nc.dram_tensor("bvecs", [4, P, S * C], F32, kind="ExternalInput")
    wj2 = nc.dram_tensor("wj2", [JOIN, V], F32, kind="ExternalInput")
    we = nc.dram_tensor("we", [ENC, JOIN], F32, kind="ExternalInput")
    bj2 = nc.dram_tensor("bj2", [V], F32, kind="ExternalInput")
    out = nc.dram_tensor("out", [S * NG, P, GROUP * P], F32, kind="ExternalOutput")

    # enc-slice supergroups of up to 4 slots (shared N=512 matmul streams)
    SG = [list(range(i, min(i + 4, S))) for i in range(0, S, 4)]

    with tile.TileContext(nc) as tc:
        with (
            tc.tile_pool(name="consts", bufs=1) as consts,
            tc.tile_pool(name="encTp", bufs=8) as encTp,
            tc.tile_pool(name="encpp", bufs=2) as encpp,
            tc.tile_pool(name="hp", bufs=4) as hp,
            tc.tile_pool(name="outp", bufs=4) as outp,
            tc.tile_pool(name="pse", bufs=4, space="PSUM") as pse,
            tc.tile_pool(name="psj", bufs=3, space="PSUM") as psj,
        ):
            we_sb = []
            for ec in range(4):
                t = consts.tile([P, JOIN], F32, tag=f"we{ec}")
                nc.sync.dma_start(t[:], we[ec * P : (ec + 1) * P, :])
                we_sb.append(t)
            wj2_sb = []
            for jc in range(4):
                t = consts.tile([P, V], F32, tag=f"wj2{jc}")
                nc.sync.dma_start(t[:], wj2[jc * P : (jc + 1) * P, :])
                wj2_sb.append(t)
            bv_sb = []
            for jc in range(4):
                t = consts.tile([P, S * C], F32, tag=f"bv{jc}")
                nc.sync.dma_start(t[:], bvecs[jc])
                bv_sb.append(t)
            bj2_sb = consts.tile([P, 1], F32, tag="bj2")
            nc.sync.dma_start(bj2_sb[:], bj2.rearrange("(o p) -> p o", p=P))

            # round-robin spreader for vector-ish work: DVE is ~2.3x faster
            # per op than ACT here, so send ~2 of every 7 ops to ACT.
            rr = [0]

            def h_op(dst, src, bias_ap):
                rr[0] = (rr[0] + 1) % 7
                if rr[0] < 2:
                    nc.scalar.activation(
                        dst, src, mybir.ActivationFunctionType.Relu,
                        bias=bias_ap, scale=1.0,
                    )
                else:
                    nc.vector.tensor_scalar(
                        dst, src, bias_ap, 0.0,
                        mybir.AluOpType.add, mybir.AluOpType.max,
                    )

            gi = 0
            for sg in SG:
                W = len(sg) * P
                # load this supergroup's enc slices: [e-chunk][128, W]
                encT4 = []
                for ec in range(4):
                    t = encTp.tile([P, 4 * P], F32, tag="encT4")
                    for k, s in enumerate(sg):
                        nc.sync.dma_start(
                            t[:, k * P : (k + 1) * P],
                            encT[s, ec * P : (ec + 1) * P, :],
                        )
                    encT4.append(t)
                # encp_T[jc][j, (slot, t)] = We.T @ encT
                encp4 = []
                for jc in range(4):
                    ps = pse.tile([P, 4 * P], F32, tag="pse")
                    for ec in range(4):
                        nc.tensor.matmul(
                            ps[:, :W],
                            we_sb[ec][:, jc * P : (jc + 1) * P],
                            encT4[ec][:, :W],
                            start=(ec == 0),
                            stop=(ec == 3),
                        )
                    sb = encpp.tile([P, 4 * P], F32, tag=f"encp{jc}")
                    if jc % 2 == 0:
                        nc.vector.tensor_copy(sb[:, :W], ps[:, :W])
                    else:
                        nc.scalar.copy(sb[:, :W], ps[:, :W])
                    encp4.append(sb)

                for k, s in enumerate(sg):
                    for g in range(NG):
                        # h for the 4 items of this group, batched per jc
                        h4 = []
                        for jc in range(4):
                            ht = hp.tile([P, GROUP * P], F32, tag=f"h{jc}")
                            for ci in range(GROUP):
                                c = g * GROUP + ci
                                idx = s * C + c
                                h_op(
                                    ht[:, ci * P : (ci + 1) * P],
                                    encp4[jc][:, k * P : (k + 1) * P],
                                    bv_sb[jc][:, idx : idx + 1],
                                )
                            h4.append(ht)
                        ps = psj.tile([P, GROUP * P], F32, tag="psj")
                        for jc in range(4):
                            nc.tensor.matmul(
                                ps[:],
                                wj2_sb[jc],
                                h4[jc][:],
                                start=(jc == 0),
                                stop=(jc == 3),
                            )
                        ot = outp.tile([P, GROUP * P], F32, tag="out")
                        nc.vector.tensor_scalar(
                            ot[:], ps[:], bj2_sb[:], None, mybir.AluOpType.add
                        )
                        nc.sync.dma_start(out[gi], ot[:])
                        gi += 1
    _split_excess_waits(nc)
    return nc


def _host_bvec(targets, emb, W1, b1, W2, b2, Wj1, bj1):
    """Prediction network on host -> bvec[b, u, JOIN] (pred_proj + bj1)."""
    tgt = np.asarray(targets).astype(np.int64)
    ext = np.pad(tgt, ((0, 0), (H, 0)), constant_values=V - 1)  # [B, U+H]
    ctx0 = ext[:, 1 : 1 + NU]  # ext[H-1-0 : L-0]
    ctx1 = ext[:, 0:NU]
    e = np.concatenate([emb[ctx0], emb[ctx1]], axis=-1)  # [B, NU, H*EMB]
    p = np.maximum(e @ W1 + b1, 0.0)
    pred = np.maximum(p @ W2 + b2, 0.0)  # [B, NU, PRED]
    Wp = Wj1[ENC:]
    return (pred @ Wp + bj1).astype(np.float32)  # [B, NU, JOIN]


def _schedule(enc_sizes, tgt_sizes, S, C):
    """Pack valid (b, t0, u) items into per-core (slot, item) grids.

    Returns a list of launches; each launch is a list of up to 8 cores;
    each core is a dict with 'slots' (list of (b, t0)) and 'grid'
    (S x C entries of (b, t0, u) or None)."""
    items = []
    for b in range(B):
        ttiles = max(1, math.ceil(int(enc_sizes[b]) / P))
        ucnt = int(tgt_sizes[b]) + 1
        for tt in range(ttiles):
            for u in range(ucnt):
                items.append((b, tt * P, u))
    total = len(items)

    launches = []
    pos = 0
    while pos < total:
        remaining = total - pos
        ncores = 8
        target = math.ceil(remaining / ncores)
        cores = []
        for _ in range(ncores):
            if pos >= total:
                break
            core = {"slots": [], "grid": [[None] * C for _ in range(S)]}
            count = 0
            cur_slice = None
            si = -1
            ci = 0
            while pos < total and count < target:
                b, t0, u = items[pos]
                if (b, t0) != cur_slice or ci >= C:
                    # need a new slot for this slice
                    if si + 1 >= S:
                        break  # core out of slots
                    si += 1
                    core["slots"].append((b, t0))
                    cur_slice = (b, t0)
                    ci = 0
                core["grid"][si][ci] = (b, t0, u)
                ci += 1
                count += 1
                pos += 1
            cores.append(core)
        launches.append(cores)
    return launches


def _get_compiled(S, C):
    key = (S, C)
    if key not in _CACHE:
        _CACHE[key] = _build_nc(S, C)
    return _CACHE[key]


def kernel(
    encoder_states,
    encoder_states_size,
    targets,
    targets_size,
    emb,
    W1,
    b1,
    W2,
    b2,
    Wj1,
    bj1,
    Wj2,
    bj2,
):
    enc = np.ascontiguousarray(np.asarray(encoder_states, dtype=np.float32))
    enc_sizes = np.asarray(encoder_states_size).astype(np.int64)
    tgt_sizes = np.asarray(targets_size).astype(np.int64)
    emb = np.asarray(emb, dtype=np.float32)
    W1 = np.asarray(W1, dtype=np.float32)
    b1 = np.asarray(b1, dtype=np.float32)
    W2 = np.asarray(W2, dtype=np.float32)
    b2 = np.asarray(b2, dtype=np.float32)
    Wj1 = np.asarray(Wj1, dtype=np.float32)
    bj1 = np.asarray(bj1, dtype=np.float32)
    Wj2 = np.ascontiguousarray(np.asarray(Wj2, dtype=np.float32))
    bj2 = np.asarray(bj2, dtype=np.float32)

    S, C = S_SLOTS, C_ITEMS
    bvec = _host_bvec(targets, emb, W1, b1, W2, b2, Wj1, bj1)
    We = np.ascontiguousarray(Wj1[:ENC])
    launches = _schedule(enc_sizes, tgt_sizes, S, C)

    nc = _get_compiled(S, C)

    trace = bool(os.environ.get("KERNEL_TRACE"))
    if trace:
        _install_ntff_hook()

    final = np.zeros((B, T, NU, V), dtype=np.float32)
    kernel.last_results = []

    for cores in launches:
        in_maps = []
        for core in cores:
            encT_arr = np.zeros((S, ENC, P), dtype=np.float32)
            for si, (b, t0) in enumerate(core["slots"]):
                encT_arr[si] = enc[b, t0 : t0 + P, :].T
            bv_arr = np.zeros((4, P, S * C), dtype=np.float32)
            for si in range(S):
                for c in range(C):
                    it = core["grid"][si][c]
                    if it is None:
                        continue
                    b, t0, u = it
                    vec = bvec[b, u]  # [JOIN]
                    bv_arr[:, :, si * C + c] = vec.reshape(4, P)
            in_maps.append({
                "encT": encT_arr,
                "bvecs": bv_arr,
                "wj2": Wj2,
                "we": We,
                "bj2": bj2,
            })
        # pad to 8 cores (SPMD requires all 8)
        while len(in_maps) < 8:
            in_maps.append({k: np.zeros_like(v) for k, v in in_maps[0].items()})

        kwargs = {}
        if trace:
            kwargs = dict(trace=True, trace_cores=list(range(8)))
        res = bass_utils.run_bass_kernel_spmd(
            nc, in_maps, core_ids=list(range(8)), **kwargs
        )
        kernel.last_results.append(res)

        for ki, core in enumerate(cores):
            out_core = res.results[ki]["out"]  # [S*NG, 128, GROUP*128]
            for si in range(S):
                for g in range(NGROUP):
                    gi = si * NGROUP + g
                    for ci in range(GROUP):
                        it = core["grid"][si][g * GROUP + ci]
                        if it is None:
                            continue
                        b, t0, u = it
                        rows = min(P, int(enc_sizes[b]) - t0)
                        if rows <= 0:
                            continue
                        tile_vt = out_core[gi][:, ci * P : (ci + 1) * P]  # [v, t]
                        final[b, t0 : t0 + rows, u, :] = tile_vt.T[:rows]

    return final


# revision 8
# speedup vs baseline: 3.0626x; 3.0626x over previous
"""Trainium2 Bass kernel for nn_FFNNTransducerModel (RNN-T style transducer).

Strategy
--------
The output grid [B, T, U+1, V] is ragged: only t < enc_size[b], u <= tgt_size[b]
is nonzero (the reference multiplies by that mask).  We therefore:

  host:   - run the tiny prediction network (embedding + 2-layer MLP + Wp
            projection + bj1) in numpy -> per-(b,u) bias vector bvec[b,u,512]
          - enumerate valid work items (b, t_tile, u), load-balance them
            across the 8 NeuronCores into a fixed (S slots x C items) grid
          - gather each core's enc slices (transposed) and bias vectors into
            dense, uniform-shaped input arrays (SPMD: one program, per-core
            data)
  device: - enc projection:  encp_T[j, t] = We.T @ encT  (PE, N=512 streams)
          - per item:        h[jc] = relu(encp_T[jc] + bvec)   (DVE/ACT,
                             fused add+relu via tensor_scalar / activation)
          - joint GEMM:      psum[v, (4 items x t)] += wj2[jc].T @ h4[jc]
                             (4 items batched along free dim -> N=512)
          - epilogue:        out = psum + bj2 (per-partition bias), DMA out
  host:   - scatter item tiles (transposed) into the zero-initialized full
            output; invalid region stays exactly 0 like the reference.

The compiled program is input-independent (all data dependence lives in the
host-prepared arrays), so the NEFF is built once and cached.
"""

import math
import os
import sys
import types

import numpy as np

import concourse.bass as bass
import concourse.mybir as mybir
import concourse.tile as tile
from concourse import bass_utils

F32 = mybir.dt.float32
F32R = mybir.dt.float32r
BF16 = mybir.dt.bfloat16
P = 128

# Precision mode: "bf16" (fast, ~1e-3 rel err) or "f32r" (safe, ~1e-6)
PREC = os.environ.get("KERNEL_PREC", "bf16")

# Model dims (fixed by the problem)
B, T, U, V = 8, 512, 64, 128
ENC, PRED, JOIN, EMB, H = 512, 256, 512, 128, 2
NU = U + 1  # 65

# Device grid: S enc-slice slots x C items per slot, items grouped by 4
S_SLOTS = 11
C_ITEMS = 8  # must be a multiple of 4
GROUP = 4
NGROUP = C_ITEMS // GROUP  # groups per slot

_CACHE = {}


def _install_ntff_hook():
    """The image's antenv lacks axon_hooks; shim it so trace=True works."""
    if "antenv.axon_hooks" in sys.modules:
        return
    mod = types.ModuleType("antenv.axon_hooks")
    _hook = [None]
    mod.set_axon_ntff_profile_hook = lambda h: _hook.__setitem__(0, h)
    mod.get_axon_ntff_profile_hook = lambda: _hook[0]
    sys.modules["antenv.axon_hooks"] = mod
    try:
        from trn_agent_boot.trn_boot import _ntff_profile_via_ctypes

        mod.set_axon_ntff_profile_hook(
            _ntff_profile_via_ctypes("/opt/axon/libaxon_pjrt.so")
        )
    except Exception:
        pass


def _split_excess_waits(nc, max_waits=1):
    """This container's walrus supports only one embedded sync-wait per
    instruction; split extras into standalone EventSemaphore waits placed
    immediately before the consumer on the same engine stream."""
    f = nc.m.functions[0]
    for blk in f.blocks:
        insts = list(blk.instructions)
        out = []
        changed = False
        for ins in insts:
            si = getattr(ins, "sync_info", None)
            if si is not None and si.on_wait is not None and len(si.on_wait) > max_waits:
                waits = list(si.on_wait)
                keep, excess = waits[:max_waits], waits[max_waits:]
                for j, w in enumerate(excess):
                    es = mybir.InstEventSemaphore(
                        name=f"{ins.name}_xw{j}",
                        engine=ins.engine,
                        sync_info=mybir.SyncInfo(on_wait=[w], on_update=[]),
                    )
                    out.append(es)
                si.on_wait = keep
                changed = True
            out.append(ins)
        if changed:
            blk.instructions = out
    return nc


def _build_nc(S, C):
    """Uniform SPMD program: all data dependence is in the input arrays."""
    NG = C // GROUP
    nc = bass.Bass()
    MMDT = BF16 if PREC == "bf16" else F32R
    encT = nc.dram_tensor("encT", [S, ENC, P], MMDT, kind="ExternalInput")
    bvecs = nc.dram_tensor("bvecs", [4, P, S * C], F32, kind="ExternalInput")
    wj2 = nc.dram_tensor("wj2", [JOIN, V], MMDT, kind="ExternalInput")
    we = nc.dram_tensor("we", [ENC, JOIN], MMDT, kind="ExternalInput")
    bj2 = nc.dram_tensor("bj2", [V], F32, kind="ExternalInput")
    out = nc.dram_tensor("out", [S * NG, P, GROUP * P], F32, kind="ExternalOutput")

    # enc-slice supergroups of up to 4 slots (shared N=512 matmul streams)
    SG = [list(range(i, min(i + 4, S))) for i in range(0, S, 4)]

    with tile.TileContext(nc) as tc:
        with (
            tc.tile_pool(name="consts", bufs=1) as consts,
            tc.tile_pool(name="encTp", bufs=8) as encTp,
            tc.tile_pool(name="encpp", bufs=2) as encpp,
            tc.tile_pool(name="hp", bufs=4) as hp,
            tc.tile_pool(name="outp", bufs=4) as outp,
            tc.tile_pool(name="pse", bufs=4, space="PSUM") as pse,
            tc.tile_pool(name="psj", bufs=3, space="PSUM") as psj,
        ):
            we_sb = []
            for ec in range(4):
                t = consts.tile([P, JOIN], MMDT, tag=f"we{ec}")
                nc.sync.dma_start(t[:], we[ec * P : (ec + 1) * P, :])
                we_sb.append(t)
            wj2_sb = []
            for jc in range(4):
                t = consts.tile([P, V], MMDT, tag=f"wj2{jc}")
                nc.sync.dma_start(t[:], wj2[jc * P : (jc + 1) * P, :])
                wj2_sb.append(t)
            bv_sb = []
            for jc in range(4):
                t = consts.tile([P, S * C], F32, tag=f"bv{jc}")
                nc.sync.dma_start(t[:], bvecs[jc])
                bv_sb.append(t)
            bj2_sb = consts.tile([P, 1], F32, tag="bj2")
            nc.sync.dma_start(bj2_sb[:], bj2.rearrange("(o p) -> p o", p=P))

            # round-robin spreader for vector-ish work
            rr = [0]
            epi_rr = [0]

            def h_op(dst, src, bias_ap):
                rr[0] = (rr[0] + 1) % 3
                if rr[0] < 1:
                    nc.scalar.activation(
                        dst, src, mybir.ActivationFunctionType.Relu,
                        bias=bias_ap, scale=1.0,
                    )
                else:
                    nc.vector.tensor_scalar(
                        dst, src, bias_ap, 0.0,
                        mybir.AluOpType.add, mybir.AluOpType.max,
                    )

            gi = 0
            for sg in SG:
                W = len(sg) * P
                # load this supergroup's enc slices: [e-chunk][128, W]
                encT4 = []
                for ec in range(4):
                    t = encTp.tile([P, 4 * P], MMDT, tag="encT4")
                    for k, s in enumerate(sg):
                        nc.sync.dma_start(
                            t[:, k * P : (k + 1) * P],
                            encT[s, ec * P : (ec + 1) * P, :],
                        )
                    encT4.append(t)
                # encp_T[jc][j, (slot, t)] = We.T @ encT
                encp4 = []
                for jc in range(4):
                    ps = pse.tile([P, 4 * P], F32, tag="pse")
                    for ec in range(4):
                        nc.tensor.matmul(
                            ps[:, :W],
                            we_sb[ec][:, jc * P : (jc + 1) * P],
                            encT4[ec][:, :W],
                            start=(ec == 0),
                            stop=(ec == 3),
                        )
                    sb = encpp.tile([P, 4 * P], MMDT if PREC == "bf16" else F32, tag=f"encp{jc}")
                    if jc % 2 == 0:
                        nc.vector.tensor_copy(sb[:, :W], ps[:, :W])
                    else:
                        nc.scalar.copy(sb[:, :W], ps[:, :W])
                    encp4.append(sb)

                for k, s in enumerate(sg):
                    for g in range(NG):
                        # h for the 4 items of this group, batched per jc
                        h4 = []
                        for jc in range(4):
                            ht = hp.tile([P, GROUP * P], MMDT, tag=f"h{jc}")
                            for ci in range(GROUP):
                                c = g * GROUP + ci
                                idx = s * C + c
                                h_op(
                                    ht[:, ci * P : (ci + 1) * P],
                                    encp4[jc][:, k * P : (k + 1) * P],
                                    bv_sb[jc][:, idx : idx + 1],
                                )
                            h4.append(ht)
                        ps = psj.tile([P, GROUP * P], F32, tag="psj")
                        for jc in range(4):
                            nc.tensor.matmul(
                                ps[:],
                                wj2_sb[jc][:],
                                h4[jc][:],
                                start=(jc == 0),
                                stop=(jc == 3),
                            )
                        ot = outp.tile([P, GROUP * P], F32, tag="out")
                        epi_rr[0] ^= 1
                        if epi_rr[0]:
                            nc.scalar.activation(
                                ot[:], ps[:], mybir.ActivationFunctionType.Identity,
                                bias=bj2_sb[:], scale=1.0,
                            )
                        else:
                            nc.vector.tensor_scalar(
                                ot[:], ps[:], bj2_sb[:], None, mybir.AluOpType.add
                            )
                        nc.sync.dma_start(out[gi], ot[:])
                        gi += 1
    _split_excess_waits(nc)
    return nc


def _host_bvec(targets, emb, W1, b1, W2, b2, Wj1, bj1):
    """Prediction network on host -> bvec[b, u, JOIN] (pred_proj + bj1)."""
    tgt = np.asarray(targets).astype(np.int64)
    ext = np.pad(tgt, ((0, 0), (H, 0)), constant_values=V - 1)  # [B, U+H]
    ctx0 = ext[:, 1 : 1 + NU]  # ext[H-1-0 : L-0]
    ctx1 = ext[:, 0:NU]
    e = np.concatenate([emb[ctx0], emb[ctx1]], axis=-1)  # [B, NU, H*EMB]
    p = np.maximum(e @ W1 + b1, 0.0)
    pred = np.maximum(p @ W2 + b2, 0.0)  # [B, NU, PRED]
    Wp = Wj1[ENC:]
    return (pred @ Wp + bj1).astype(np.float32)  # [B, NU, JOIN]


def _schedule(enc_sizes, tgt_sizes, S, C):
    """Pack valid (b, t0, u-range) slices into per-core (slot, item) grids.

    LPT-style bin packing over 8 cores with both an item capacity (S*C)
    and a slot capacity (S); slices split across cores when needed.
    Returns a list of launches; each launch is a list of up to 8 cores;
    each core is a dict with 'slots' (list of (b, t0)) and 'grid'
    (S x C entries of (b, t0, u) or None)."""
    slices = []
    for b in range(B):
        ttiles = max(1, math.ceil(int(enc_sizes[b]) / P))
        ucnt = int(tgt_sizes[b]) + 1
        for tt in range(ttiles):
            slices.append((b, tt * P, ucnt))
    slices.sort(key=lambda s: -s[2])

    launches = []
    pending = [(b, t0, 0, n) for (b, t0, n) in slices]  # (b, t0, u0, count)
    while pending:
        cores = [
            {"items": 0, "nslots": 0, "slots": [], "grid": [[None] * C for _ in range(S)]}
            for _ in range(8)
        ]
        overflow = []
        for b, t0, u0, left in pending:
            while left > 0:
                order = sorted(range(8), key=lambda i: cores[i]["items"])
                placed = False
                for i in order:
                    c = cores[i]
                    cap = (S - c["nslots"]) * C
                    if cap <= 0:
                        continue
                    take = min(left, cap)
                    nslots = math.ceil(take / C)
                    for j in range(take):
                        si = c["nslots"] + j // C
                        ci = j % C
                        c["grid"][si][ci] = (b, t0, u0 + j)
                    for _ in range(nslots):
                        c["slots"].append((b, t0))
                    c["nslots"] += nslots
                    c["items"] += take
                    u0 += take
                    left -= take
                    placed = True
                    break
                if not placed:
                    overflow.append((b, t0, u0, left))
                    break
        launches.append([c for c in cores if c["items"] > 0])
        pending = overflow
    return launches


def _get_compiled(S, C):
    key = (S, C)
    if key not in _CACHE:
        _CACHE[key] = _build_nc(S, C)
    return _CACHE[key]


def kernel(
    encoder_states,
    encoder_states_size,
    targets,
    targets_size,
    emb,
    W1,
    b1,
    W2,
    b2,
    Wj1,
    bj1,
    Wj2,
    bj2,
):
    enc = np.ascontiguousarray(np.asarray(encoder_states, dtype=np.float32))
    enc_sizes = np.asarray(encoder_states_size).astype(np.int64)
    tgt_sizes = np.asarray(targets_size).astype(np.int64)
    emb = np.asarray(emb, dtype=np.float32)
    W1 = np.asarray(W1, dtype=np.float32)
    b1 = np.asarray(b1, dtype=np.float32)
    W2 = np.asarray(W2, dtype=np.float32)
    b2 = np.asarray(b2, dtype=np.float32)
    Wj1 = np.asarray(Wj1, dtype=np.float32)
    bj1 = np.asarray(bj1, dtype=np.float32)
    Wj2 = np.ascontiguousarray(np.asarray(Wj2, dtype=np.float32))
    bj2 = np.asarray(bj2, dtype=np.float32)

    S, C = S_SLOTS, C_ITEMS
    bvec = _host_bvec(targets, emb, W1, b1, W2, b2, Wj1, bj1)
    import ml_dtypes
    mmdt = ml_dtypes.bfloat16 if PREC == "bf16" else np.float32
    We = np.ascontiguousarray(Wj1[:ENC]).astype(mmdt)
    launches = _schedule(enc_sizes, tgt_sizes, S, C)

    nc = _get_compiled(S, C)

    trace = bool(os.environ.get("KERNEL_TRACE"))
    if trace:
        _install_ntff_hook()

    final = np.zeros((B, T, NU, V), dtype=np.float32)
    kernel.last_results = []

    for cores in launches:
        in_maps = []
        for core in cores:
            encT_arr = np.zeros((S, ENC, P), dtype=mmdt)
            for si, (b, t0) in enumerate(core["slots"]):
                encT_arr[si] = enc[b, t0 : t0 + P, :].T.astype(mmdt)
            bv_arr = np.zeros((4, P, S * C), dtype=np.float32)
            for si in range(S):
                for c in range(C):
                    it = core["grid"][si][c]
                    if it is None:
                        continue
                    b, t0, u = it
                    vec = bvec[b, u]  # [JOIN]
                    bv_arr[:, :, si * C + c] = vec.reshape(4, P)
            in_maps.append({
                "encT": encT_arr,
                "bvecs": bv_arr,
                "wj2": Wj2.astype(mmdt),
                "we": We,
                "bj2": bj2,
            })
        # pad to 8 cores (SPMD requires all 8)
        while len(in_maps) < 8:
            in_maps.append({k: np.zeros_like(v) for k, v in in_maps[0].items()})

        kwargs = {}
        if trace:
            kwargs = dict(trace=True, trace_cores=list(range(8)))
        res = bass_utils.run_bass_kernel_spmd(
            nc, in_maps, core_ids=list(range(8)), **kwargs
        )
        kernel.last_results.append(res)

        for ki, core in enumerate(cores):
            out_core = res.results[ki]["out"]  # [S*NG, 128, GROUP*128]
            for si in range(S):
                for g in range(NGROUP):
                    gi = si * NGROUP + g
                    for ci in range(GROUP):
                        it = core["grid"][si][g * GROUP + ci]
                        if it is None:
                            continue
                        b, t0, u = it
                        rows = min(P, int(enc_sizes[b]) - t0)
                        if rows <= 0:
                            continue
                        tile_vt = out_core[gi][:, ci * P : (ci + 1) * P]  # [v, t]
                        final[b, t0 : t0 + rows, u, :] = tile_vt.T[:rows]

    return final


# revision 19
# speedup vs baseline: 4.8097x; 1.5705x over previous
"""Trainium2 Bass kernel for nn_FFNNTransducerModel (RNN-T style transducer).

Strategy
--------
The output grid [B, T, U+1, V] is ragged: only t < enc_size[b], u <= tgt_size[b]
is nonzero (the reference multiplies by that mask).  So:

  host:   - run the tiny prediction network (embedding + 2-layer MLP + Wp
            projection + bj1) in numpy -> per-(b,u) bias vector bvec[b,u,512]
          - decompose each example's valid t-tiles into width-3 and width-1
            tile chunks, load-balance (chunk, u) items across the 8 cores
            into two fixed grids (SPMD: one program, per-core data):
              section A: SA slots (3 t-tiles wide) x CA items (one u each)
              section B: SB slots (1 t-tile)      x CB items, grouped by 4
          - any overflow beyond grid capacity is computed on the host
  device: - enc projection per slot: encp_T[j, t] = We.T @ encT  (PE)
          - per item: h[jc] = relu(encp_T[jc] + bvec[u])  (DVE/ACT fused
            tensor_scalar / activation, FD = slot width * 128)
          - joint GEMM: psum[v, t*] += wj2[jc].T @ h[jc]  (fp32 PSUM accum)
          - epilogue: out = psum + bj2 (per-partition bias on ACT), DMA out
  host:   - scatter item tiles (transposed) into the zero-initialized output;
            the invalid region stays exactly 0 like the reference.

Matmul operands run in bf16 by default (KERNEL_PREC=f32r for the ~1e-6 exact
mode); fp32 PSUM accumulation.  The compiled program depends only on the grid
shape, which is derived from the input sizes and cached.
"""

import math
import os
import sys
import types

import numpy as np

import concourse.bass as bass
import concourse.mybir as mybir
import concourse.tile as tile
from concourse import bass_utils

F32 = mybir.dt.float32
F32R = mybir.dt.float32r
BF16 = mybir.dt.bfloat16
P = 128

# Model dims (fixed by the problem)
B, T, U, V = 8, 512, 64, 128
ENC, PRED, JOIN, EMB, H = 512, 256, 512, 128, 2
NU = U + 1  # 65

# Precision mode: "bf16" (fast, ~4e-3 rel err) or "f32r" (safe, ~1e-6)
PREC = os.environ.get("KERNEL_PREC", "bf16")

_CACHE = {}


def _install_ntff_hook():
    """The image's antenv lacks axon_hooks; shim it so trace=True works."""
    if "antenv.axon_hooks" in sys.modules:
        return
    mod = types.ModuleType("antenv.axon_hooks")
    _hook = [None]
    mod.set_axon_ntff_profile_hook = lambda h: _hook.__setitem__(0, h)
    mod.get_axon_ntff_profile_hook = lambda: _hook[0]
    sys.modules["antenv.axon_hooks"] = mod
    try:
        from trn_agent_boot.trn_boot import _ntff_profile_via_ctypes

        mod.set_axon_ntff_profile_hook(
            _ntff_profile_via_ctypes("/opt/axon/libaxon_pjrt.so")
        )
    except Exception:
        pass


def _split_excess_waits(nc, max_waits=1):
    """This container's walrus supports only one embedded sync-wait per
    instruction; split extras into standalone EventSemaphore waits placed
    immediately before the consumer on the same engine stream."""
    f = nc.m.functions[0]
    for blk in f.blocks:
        insts = list(blk.instructions)
        out = []
        changed = False
        for ins in insts:
            si = getattr(ins, "sync_info", None)
            if si is not None and si.on_wait is not None and len(si.on_wait) > max_waits:
                waits = list(si.on_wait)
                keep, excess = waits[:max_waits], waits[max_waits:]
                for j, w in enumerate(excess):
                    es = mybir.InstEventSemaphore(
                        name=f"{ins.name}_xw{j}",
                        engine=ins.engine,
                        sync_info=mybir.SyncInfo(on_wait=[w], on_update=[]),
                    )
                    out.append(es)
                si.on_wait = keep
                changed = True
            out.append(ins)
        if changed:
            blk.instructions = out
    return nc


def _build_nc(SA, CA, SB, CB):
    """Uniform SPMD program; all data dependence lives in the input arrays.

    Section A: SA slots of width WA=3 t-tiles, CA items (u values) each.
    Section B: SB slots of width 1 t-tile, CB items each, grouped by 4
    into one PSUM bank (CB % 4 == 0)."""
    WA = 3
    MMDT = BF16 if PREC == "bf16" else F32R
    EVDT = MMDT  # encp / h storage dtype
    NITA = SA * CA
    NGB = (SB * CB) // 4 if SB else 0

    nc = bass.Bass()
    encA = nc.dram_tensor("encA", [SA, 4, P, WA * P], MMDT, kind="ExternalInput")
    bvA = nc.dram_tensor("bvA", [4, P, max(NITA, 1)], F32, kind="ExternalInput")
    if SB:
        encB = nc.dram_tensor("encB", [4, P, SB * P], MMDT, kind="ExternalInput")
        bvB = nc.dram_tensor("bvB", [4, P, SB * CB], F32, kind="ExternalInput")
    wj2 = nc.dram_tensor("wj2", [JOIN, V], MMDT, kind="ExternalInput")
    we = nc.dram_tensor("we", [ENC, JOIN], MMDT, kind="ExternalInput")
    bj2 = nc.dram_tensor("bj2", [V], F32, kind="ExternalInput")
    NPAIR = max(NITA, 1) // 2 if NITA % 2 == 0 and NITA > 0 else max(NITA, 1)
    PAIRED = NITA > 0 and NITA % 2 == 0
    outA = nc.dram_tensor(
        "outA",
        [NPAIR, P, (2 if PAIRED else 1) * WA * P],
        F32,
        kind="ExternalOutput",
    )
    if SB:
        outB = nc.dram_tensor("outB", [NGB, P, 4 * P], F32, kind="ExternalOutput")

    with tile.TileContext(nc) as tc:
        with (
            tc.tile_pool(name="consts", bufs=1) as consts,
            tc.tile_pool(name="encTp", bufs=8) as encTp,
            tc.tile_pool(name="encpp", bufs=3) as encpp,
            tc.tile_pool(name="hp", bufs=8) as hp,
            tc.tile_pool(name="outp", bufs=6) as outp,
            tc.tile_pool(name="pse", bufs=4, space="PSUM") as pse,
            tc.tile_pool(name="psj", bufs=4, space="PSUM") as psj,
        ):
            we_all = consts.tile([P, 4 * JOIN], MMDT, tag="we")
            nc.sync.dma_start(
                we_all[:].rearrange("p (ec j) -> p ec j", ec=4),
                we.rearrange("(ec p) j -> p ec j", p=P),
            )
            we_sb = [we_all[:, ec * JOIN : (ec + 1) * JOIN] for ec in range(4)]
            wj2_all = consts.tile([P, 4 * V], MMDT, tag="wj2")
            nc.sync.dma_start(
                wj2_all[:].rearrange("p (jc v) -> p jc v", jc=4),
                wj2.rearrange("(jc p) v -> p jc v", p=P),
            )
            wj2_sb = [wj2_all[:, jc * V : (jc + 1) * V] for jc in range(4)]
            NA = max(NITA, 1)
            bvA_all = consts.tile([P, 4 * NA], F32, tag="bvA")
            nc.sync.dma_start(
                bvA_all[:].rearrange("p (jc n) -> p jc n", jc=4),
                bvA.rearrange("jc p n -> p jc n"),
            )
            bvA_sb = [bvA_all[:, jc * NA : (jc + 1) * NA] for jc in range(4)]
            if SB:
                bvB_all = consts.tile([P, 4 * SB * CB], F32, tag="bvB")
                nc.sync.dma_start(
                    bvB_all[:].rearrange("p (jc n) -> p jc n", jc=4),
                    bvB.rearrange("jc p n -> p jc n"),
                )
                bvB_sb = [bvB_all[:, jc * SB * CB : (jc + 1) * SB * CB] for jc in range(4)]
            bj2_sb = consts.tile([P, 1], F32, tag="bj2")
            nc.sync.dma_start(bj2_sb[:], bj2.rearrange("(o p) -> p o", p=P))

            # ~2 of 9 h-ops go to ACT (DVE is faster per op; ACT also owns
            # the epilogue + encp evacuation)
            rr = [0]

            def h_op(dst, src, bias_ap):
                rr[0] = (rr[0] + 1) % 7
                if rr[0] < 1:
                    nc.scalar.activation(
                        dst, src, mybir.ActivationFunctionType.Relu,
                        bias=bias_ap, scale=1.0,
                    )
                else:
                    nc.vector.tensor_scalar(
                        dst, src, bias_ap, 0.0,
                        mybir.AluOpType.add, mybir.AluOpType.max,
                    )

            def enc_proj(dram_ap, width):
                """Project one enc slice set: returns encp[jc] [P, width].

                dram_ap: [4, P, width] (e-chunk, partition, t) -- loaded
                with a single DMA into [P, (ec, t)]."""
                tall = encTp.tile([P, 4 * WA * P], MMDT, tag="encT4")
                nc.sync.dma_start(
                    tall[:, : 4 * width].rearrange("p (ec t) -> p ec t", ec=4),
                    dram_ap.rearrange("ec p t -> p ec t"),
                )
                encT4 = [tall[:, ec * width : (ec + 1) * width] for ec in range(4)]
                encp4 = []
                for jc in range(4):
                    ps = pse.tile([P, WA * P], F32, tag="pse")
                    for ec in range(4):
                        nc.tensor.matmul(
                            ps[:, :width],
                            we_sb[ec][:, jc * P : (jc + 1) * P],
                            encT4[ec],
                            start=(ec == 0),
                            stop=(ec == 3),
                        )
                    sb = encpp.tile([P, WA * P], EVDT, tag=f"encp{jc}")
                    if jc % 2 == 0:
                        nc.vector.tensor_copy(sb[:, :width], ps[:, :width])
                    else:
                        nc.scalar.copy(sb[:, :width], ps[:, :width])
                    encp4.append(sb)
                return encp4

            # ---- section A: width-3 slots ----
            encp_cur = enc_proj(encA[0], WA * P) if SA else None
            for s in range(SA):
                encp_next = None
                for c in range(CA):
                    if c == min(2, CA - 1) and s + 1 < SA:
                        encp_next = enc_proj(encA[s + 1], WA * P)
                    idx = s * CA + c
                    h4 = []
                    for jc in range(4):
                        ht = hp.tile([P, WA * P], EVDT, tag=f"h{jc}")
                        h_op(ht[:], encp_cur[jc][:], bvA_sb[jc][:, idx : idx + 1])
                        h4.append(ht)
                    ps = psj.tile([P, WA * P], F32, tag="psj")
                    for jc in range(4):
                        nc.tensor.matmul(
                            ps[:], wj2_sb[jc], h4[jc][:],
                            start=(jc == 0), stop=(jc == 3),
                        )
                    if PAIRED:
                        if idx % 2 == 0:
                            ot_pair = outp.tile([P, 2 * WA * P], F32, tag="out")
                        half = idx % 2
                        dst = ot_pair[:, half * WA * P : (half + 1) * WA * P]
                        nc.scalar.activation(
                            dst, ps[:], mybir.ActivationFunctionType.Identity,
                            bias=bj2_sb[:], scale=1.0,
                        )
                        if half == 1:
                            nc.sync.dma_start(outA[idx // 2], ot_pair[:])
                    else:
                        ot = outp.tile([P, WA * P], F32, tag="out")
                        nc.scalar.activation(
                            ot[:], ps[:], mybir.ActivationFunctionType.Identity,
                            bias=bj2_sb[:], scale=1.0,
                        )
                        nc.sync.dma_start(outA[idx], ot[:])
                if encp_next is not None:
                    encp_cur = encp_next

            # ---- section B: width-1 slots, items grouped by 4 per PSUM bank ----
            if SB:
                WB = SB * P
                tallb = encTp.tile([P, 4 * WA * P], MMDT, tag="encT4")
                nc.sync.dma_start(
                    tallb[:, : 4 * WB].rearrange("p (ec t) -> p ec t", ec=4),
                    encB.rearrange("ec p t -> p ec t"),
                )
                encT4b = [tallb[:, ec * WB : (ec + 1) * WB] for ec in range(4)]
                encpb = []
                for jc in range(4):
                    ps = pse.tile([P, WA * P], F32, tag="pse")
                    for ec in range(4):
                        nc.tensor.matmul(
                            ps[:, :WB],
                            we_sb[ec][:, jc * P : (jc + 1) * P],
                            encT4b[ec],
                            start=(ec == 0),
                            stop=(ec == 3),
                        )
                    sb = encpp.tile([P, WA * P], EVDT, tag=f"encp{jc}")
                    nc.scalar.copy(sb[:, :WB], ps[:, :WB])
                    encpb.append(sb)
                for s in range(SB):
                    for g in range(CB // 4):
                        h4 = []
                        for jc in range(4):
                            ht = hp.tile([P, 4 * P], EVDT, tag=f"h{jc}")
                            for ci in range(4):
                                c = g * 4 + ci
                                idx = s * CB + c
                                h_op(
                                    ht[:, ci * P : (ci + 1) * P],
                                    encpb[jc][:, s * P : (s + 1) * P],
                                    bvB_sb[jc][:, idx : idx + 1],
                                )
                            h4.append(ht)
                        ps = psj.tile([P, 4 * P], F32, tag="psj")
                        for jc in range(4):
                            nc.tensor.matmul(
                                ps[:, : 4 * P], wj2_sb[jc], h4[jc][:, : 4 * P],
                                start=(jc == 0), stop=(jc == 3),
                            )
                        ot = outp.tile([P, 4 * P], F32, tag="out")
                        nc.scalar.activation(
                            ot[:, : 4 * P], ps[:, : 4 * P],
                            mybir.ActivationFunctionType.Identity,
                            bias=bj2_sb[:], scale=1.0,
                        )
                        gi = s * (CB // 4) + g
                        nc.sync.dma_start(outB[gi], ot[:, : 4 * P])
    _split_excess_waits(nc)
    return nc


def _host_bvec(targets, emb, W1, b1, W2, b2, Wj1, bj1):
    """Prediction network on host -> bvec[b, u, JOIN] (pred_proj + bj1)."""
    tgt = np.asarray(targets).astype(np.int64)
    ext = np.pad(tgt, ((0, 0), (H, 0)), constant_values=V - 1)  # [B, U+H]
    ctx0 = ext[:, 1 : 1 + NU]
    ctx1 = ext[:, 0:NU]
    e = np.concatenate([emb[ctx0], emb[ctx1]], axis=-1)  # [B, NU, H*EMB]
    p = np.maximum(e @ W1 + b1, 0.0)
    pred = np.maximum(p @ W2 + b2, 0.0)  # [B, NU, PRED]
    Wp = Wj1[ENC:]
    return (pred @ Wp + bj1).astype(np.float32)  # [B, NU, JOIN]


def _schedule(enc_sizes, tgt_sizes):
    """Decompose the ragged grid into width-3 / width-1 chunk work and
    LPT-pack it onto 8 cores.  Returns (SA, CA, SB, CB, cores, leftover):
    cores[i] = {"aslots": [(b,t0,w)], "agrid": [[item or None]*CA]*SA,
                "bslots": [(b,t0,w)], "bgrid": ...}; item = (b, t0, w, u);
    leftover = [(b, t0, w, u)] to compute on the host."""
    w3, w1 = [], []  # chunks: (b, t0, width, ucnt)
    for b in range(B):
        ttiles = max(1, math.ceil(int(enc_sizes[b]) / P))
        ucnt = int(tgt_sizes[b]) + 1
        t = 0
        while ttiles - t >= 3:
            w3.append((b, t * P, 3, ucnt))
            t += 3
        rem = ttiles - t
        if rem == 2:
            w3.append((b, t * P, 2, ucnt))  # padded into a width-3 slot
        elif rem == 1:
            w1.append((b, t * P, 1, ucnt))

    n3 = sum(c[3] for c in w3)
    n1 = sum(c[3] for c in w1)
    CA = 6
    SA = max(1, math.ceil((n3 / 8) / CA) + 1)
    CB = 4
    SB = max(0, math.ceil((n1 / 8) / CB) + 1) if n1 else 0

    def pack(chunks, S, C):
        cores = [
            {"slots": [], "grid": [[None] * C for _ in range(S)], "items": 0}
            for _ in range(8)
        ]
        leftover = []
        for b, t0, w, n in sorted(chunks, key=lambda c: -c[3]):
            u0 = 0
            left = n
            while left > 0:
                order = sorted(range(8), key=lambda i: cores[i]["items"])
                placed = False
                for i in order:
                    cc = cores[i]
                    cap = (S - len(cc["slots"])) * C
                    if cap <= 0:
                        continue
                    take = min(left, cap)
                    nslots = math.ceil(take / C)
                    base = len(cc["slots"])
                    for j in range(take):
                        si = base + j // C
                        cc["grid"][si][j % C] = (b, t0, w, u0 + j)
                    for _ in range(nslots):
                        cc["slots"].append((b, t0, w))
                    cc["items"] += take
                    u0 += take
                    left -= take
                    placed = True
                    break
                if not placed:
                    for j in range(left):
                        leftover.append((b, t0, w, u0 + j))
                    break
        return cores, leftover

    acores, aleft = pack(w3, SA, CA)
    if SB:
        bcores, bleft = pack(w1, SB, CB)
    else:
        bcores = [{"slots": [], "grid": [], "items": 0} for _ in range(8)]
        bleft = []
    cores = []
    for i in range(8):
        cores.append({
            "aslots": acores[i]["slots"], "agrid": acores[i]["grid"],
            "bslots": bcores[i]["slots"], "bgrid": bcores[i]["grid"],
        })
    return SA, CA, SB, CB, cores, aleft + bleft


def _get_compiled(key):
    if key not in _CACHE:
        _CACHE[key] = _build_nc(*key)
    return _CACHE[key]


def kernel(
    encoder_states,
    encoder_states_size,
    targets,
    targets_size,
    emb,
    W1,
    b1,
    W2,
    b2,
    Wj1,
    bj1,
    Wj2,
    bj2,
):
    import ml_dtypes

    enc = np.ascontiguousarray(np.asarray(encoder_states, dtype=np.float32))
    enc_sizes = np.asarray(encoder_states_size).astype(np.int64)
    tgt_sizes = np.asarray(targets_size).astype(np.int64)
    emb = np.asarray(emb, dtype=np.float32)
    W1 = np.asarray(W1, dtype=np.float32)
    b1 = np.asarray(b1, dtype=np.float32)
    W2 = np.asarray(W2, dtype=np.float32)
    b2 = np.asarray(b2, dtype=np.float32)
    Wj1 = np.asarray(Wj1, dtype=np.float32)
    bj1 = np.asarray(bj1, dtype=np.float32)
    Wj2 = np.ascontiguousarray(np.asarray(Wj2, dtype=np.float32))
    bj2 = np.asarray(bj2, dtype=np.float32)

    mmdt = ml_dtypes.bfloat16 if PREC == "bf16" else np.float32
    bvec = _host_bvec(targets, emb, W1, b1, W2, b2, Wj1, bj1)
    We = np.ascontiguousarray(Wj1[:ENC])
    SA, CA, SB, CB, cores, leftover = _schedule(enc_sizes, tgt_sizes)
    WA = 3

    nc = _get_compiled((SA, CA, SB, CB))

    trace = bool(os.environ.get("KERNEL_TRACE"))
    if trace:
        _install_ntff_hook()

    # transposed enc, cast once: encTc[b] = enc[b].T  [ENC, T]
    encTc = np.ascontiguousarray(enc.transpose(0, 2, 1)).astype(mmdt)
    We_c = We.astype(mmdt)
    Wj2_c = Wj2.astype(mmdt)

    NITA = SA * CA
    in_maps = []
    for core in cores:
        encA_arr = np.zeros((SA, 4, P, WA * P), dtype=mmdt)
        for si, (b, t0, w) in enumerate(core["aslots"]):
            wid = w * P
            for ec in range(4):
                encA_arr[si, ec, :, :wid] = encTc[b, ec * P : (ec + 1) * P, t0 : t0 + wid]
        bvA_arr = np.zeros((4, P, max(NITA, 1)), dtype=np.float32)
        for si in range(SA):
            for c in range(CA):
                it = core["agrid"][si][c]
                if it is None:
                    continue
                b, t0, w, u = it
                bvA_arr[:, :, si * CA + c] = bvec[b, u].reshape(4, P)
        m = {
            "encA": encA_arr,
            "bvA": bvA_arr,
            "wj2": Wj2_c,
            "we": We_c,
            "bj2": bj2,
        }
        if SB:
            encB_arr = np.zeros((4, P, SB * P), dtype=mmdt)
            for si, (b, t0, w) in enumerate(core["bslots"]):
                for ec in range(4):
                    encB_arr[ec, :, si * P : (si + 1) * P] = encTc[
                        b, ec * P : (ec + 1) * P, t0 : t0 + P
                    ]
            bvB_arr = np.zeros((4, P, SB * CB), dtype=np.float32)
            for si in range(SB):
                for c in range(CB):
                    it = core["bgrid"][si][c]
                    if it is None:
                        continue
                    b, t0, w, u = it
                    bvB_arr[:, :, si * CB + c] = bvec[b, u].reshape(4, P)
            m["encB"] = encB_arr
            m["bvB"] = bvB_arr
        in_maps.append(m)

    kwargs = {}
    if trace:
        kwargs = dict(trace=True, trace_cores=list(range(8)))
    res = None
    last_exc = None
    for attempt in range(3):
        try:
            res = bass_utils.run_bass_kernel_spmd(
                nc, in_maps, core_ids=list(range(8)), **kwargs
            )
            break
        except Exception as e:  # transient device wedges happen; retry
            last_exc = e
            import time as _time

            _time.sleep(2.0)
    if res is None:
        raise last_exc
    kernel.last_results = [res]

    final = np.zeros((B, T, NU, V), dtype=np.float32)
    for ki, core in enumerate(cores):
        outA = res.results[ki]["outA"]
        if NITA % 2 == 0 and NITA > 0:
            outA = outA.reshape(NITA // 2, P, 2, WA * P).transpose(0, 2, 1, 3).reshape(
                NITA, P, WA * P
            )
        for si in range(SA):
            for c in range(CA):
                it = core["agrid"][si][c]
                if it is None:
                    continue
                b, t0, w, u = it
                rows = min(w * P, int(enc_sizes[b]) - t0)
                if rows <= 0:
                    continue
                final[b, t0 : t0 + rows, u, :] = outA[si * CA + c, :, :rows].T
        if SB:
            outB = res.results[ki]["outB"]  # [NGB, 128, 512]
            for si in range(SB):
                for c in range(CB):
                    it = core["bgrid"][si][c]
                    if it is None:
                        continue
                    b, t0, w, u = it
                    rows = min(P, int(enc_sizes[b]) - t0)
                    if rows <= 0:
                        continue
                    gi = si * (CB // 4) + c // 4
                    ci = c % 4
                    final[b, t0 : t0 + rows, u, :] = outB[
                        gi, :, ci * P : ci * P + rows
                    ].T

    # host fallback for anything that didn't fit the device grids
    for b, t0, w, u in leftover:
        rows = min(w * P, int(enc_sizes[b]) - t0)
        if rows <= 0:
            continue
        ep = enc[b, t0 : t0 + rows, :] @ We
        hh = np.maximum(ep + bvec[b, u], 0.0)
        final[b, t0 : t0 + rows, u, :] = hh @ Wj2 + bj2

    return final
